# revision 1
# baseline (speedup 1.0000x reference)
# Bass/Trainium2 kernel for nn_MENet (scatter_memory).
#
# Strategy: pure data parallel over batch (512 -> 64 per core, 8 cores).
# Host pre-folds BN scales into weights, fuses mlp_w1 @ memory_w.T (so the
# [B,64,32] memory read-out is never materialized), permutes fc1 weight
# columns to match the on-chip maxpool layout, and packs all weights into a
# single [128, NW] tensor already in SBUF layout.
#
# On chip (per core):
#   - l3_points/x2_points row maxes: streamed [128, 2, 8, C] tiles (4/8KB
#     contiguous per partition), one segmented DVE reduce per 2 batches.
#   - memory addressing: stacked [values; squares] SBUF tile so ONE K=128
#     matmul per 128 (b,n) rows yields both logits (N=16) and sum-of-squares
#     (col 16); softmax on DVE/ACT; attention rows transposed via PE into the
#     fused MLP, ReLU+bias on ACT (per-partition bias), segmented max into
#     head-input layout.
#   - heads: small PE matmuls (K-chunks of 128) + ACT bias/ReLU; log_softmax
#     via PE transpose + Exp-with-accum + Ln.
import os
from contextlib import ExitStack

import numpy as np

import concourse.bacc as bacc
import concourse.bass as bass
import concourse.tile as tile
from concourse import mybir
from concourse.bass_utils import run_bass_kernel_spmd

F32 = mybir.dt.float32
AF = mybir.ActivationFunctionType
ALU = mybir.AluOpType
AX = mybir.AxisListType

P = 128
NCORES = 8
B = 512
BL = B // NCORES          # 64 batches per core
NM = 32                   # n points per memory block
CM = 64                   # memory channel dim
ROWS = BL * NM            # 2048 rows per core per branch
NGROUP = ROWS // 512      # 4 groups of 512 rows (16 batches each)
EPS_BN = 1e-5


# ----------------------------------------------------------------------------
# host-side weight folding + packing
# ----------------------------------------------------------------------------
class _Pack:
    def __init__(self):
        self.parts = []
        self.off = {}
        self.pos = 0

    def add(self, name, arr):
        arr = np.asarray(arr, np.float32)
        assert arr.ndim == 2 and arr.shape[0] <= P
        buf = np.zeros((P, arr.shape[1]), np.float32)
        buf[: arr.shape[0]] = arr
        self.off[name] = (self.pos, arr.shape[1])
        self.pos += arr.shape[1]
        self.parts.append(buf)

    def finish(self):
        return np.ascontiguousarray(np.concatenate(self.parts, axis=1))


def _perm_pts(npref, npts):
    # device x-vector position npref + j*128 + q  <-  original point 8q + j
    d = np.arange(npts)
    src = npref + 8 * (d % 128) + (d // 128)
    return np.concatenate([np.arange(npref), src])


def _kpack(w_t):  # [K, M] -> [128, nk, M] flattened to [128, nk*M]
    K, M = w_t.shape
    nk = K // P
    return np.ascontiguousarray(
        np.transpose(w_t.reshape(nk, P, M), (1, 0, 2)).reshape(P, nk * M)
    )


def _fold_and_pack(f):
    s = lambda g: g / np.sqrt(1.0 + EPS_BN)
    mw = f["memory_w"]                                    # [16, 64]
    mn = mw / np.maximum(np.linalg.norm(mw, axis=1, keepdims=True), 1e-12)

    pk = _Pack()
    pk.add("ident", np.eye(P, dtype=np.float32))

    rhs2 = np.zeros((P, 17), np.float32)
    rhs2[0:CM, 0:16] = mn.T                               # logits part
    rhs2[CM:2 * CM, 16] = 1.0                             # sum-of-squares part
    pk.add("rhs2", rhs2)

    # branch mlps (conv 1x1): fold BN scale into weights, fuse layer1 with
    # memory_w read-out:  y1[o, row] = sum_s W1e[o, s] * a[row, s]
    for bi, (w1, g1, b1, w2, g2, b2) in enumerate(
        [
            (f["mlp1_w1"], f["mlp1_g1"], f["mlp1_b1"], f["mlp1_w2"], f["mlp1_g2"], f["mlp1_b2"]),
            (f["mlp2_w1"], f["mlp2_g1"], f["mlp2_b1"], f["mlp2_w2"], f["mlp2_g2"], f["mlp2_b2"]),
        ]
    ):
        w1e = (s(g1)[:, None] * w1) @ mw.T                # [M1, 16]
        w2f = s(g2)[:, None] * w2                         # [M2, M1]
        M1, M2 = w2f.shape[1], w2f.shape[0]
        pk.add(f"w1eT_b{bi + 1}", w1e.T)                  # [16, M1]
        pk.add(f"b1_b{bi + 1}", b1.reshape(M1 // P, P).T) # [128, M1/128]
        pk.add(f"w2T_b{bi + 1}", _kpack(w2f.T))           # [128, (M1/128)*M2]
        pk.add(f"b2_b{bi + 1}", b2.reshape(M2 // P, P).T)

    # heads: fold BN into fc1/fc2, permute fc1 cols for the maxpool layout
    for hi, (w1, b1, g1, bb1, w2, b2, g2, bb2, w3, b3, npref) in enumerate(
        [
            (f["fc1_w"], f["fc1_b"], f["bn1_g"], f["bn1_b"], f["fc2_w"], f["fc2_b"],
             f["bn2_g"], f["bn2_b"], f["fc3_w"], f["fc3_b"], 256),
            (f["fc1_2_w"], f["fc1_2_b"], f["bn1_2_g"], f["bn1_2_b"], f["fc2_2_w"],
             f["fc2_2_b"], f["bn2_2_g"], f["bn2_2_b"], f["fc3_2_w"], f["fc3_2_b"], 512),
        ]
    ):
        s1, s2 = s(g1), s(g2)
        w1f = (s1[:, None] * w1)[:, _perm_pts(npref, 1024)]   # [512, npref+1024]
        b1f = s1 * b1 + bb1
        w2f = s2[:, None] * w2                                # [256, 512]
        b2f = s2 * b2 + bb2
        pk.add(f"fw1_h{hi + 1}", _kpack(w1f.T))               # [128, nk1*512]
        pk.add(f"fb1_h{hi + 1}", b1f.reshape(4, P).T)
        pk.add(f"fw2_h{hi + 1}", _kpack(w2f.T))               # [128, 4*256]
        pk.add(f"fb2_h{hi + 1}", b2f.reshape(2, P).T)
        pk.add(f"fw3_h{hi + 1}", _kpack(w3.T))                # [128, 2*40]
        pk.add(f"fb3_h{hi + 1}", b3.reshape(40, 1))

    return pk.finish(), pk.off


# ----------------------------------------------------------------------------
# device program
# ----------------------------------------------------------------------------
def _build(off, NW):
    nc = bacc.Bacc("TRN2", target_bir_lowering=False, debug=False)
    l3d = nc.dram_tensor("l3", [BL, 1024, 128], F32, kind="ExternalInput").ap()
    x2d = nc.dram_tensor("x2", [BL, 1024, 256], F32, kind="ExternalInput").ap()
    mf1d = nc.dram_tensor("mf1", [CM, ROWS], F32, kind="ExternalInput").ap()
    mf2d = nc.dram_tensor("mf2", [CM, ROWS], F32, kind="ExternalInput").ap()
    wpd = nc.dram_tensor("wp", [P, NW], F32, kind="ExternalInput").ap()
    o1d = nc.dram_tensor("out1", [BL, 40], F32, kind="ExternalOutput").ap()
    o2d = nc.dram_tensor("out2", [BL, 40], F32, kind="ExternalOutput").ap()

    with tile.TileContext(nc) as tc, ExitStack() as ctx:
        pp = ctx.enter_context(tc.tile_pool(name="persist", bufs=1))
        wsb = pp.tile([P, NW], F32, name="wsb")
        nc.gpsimd.dma_start(wsb[:], wpd)

        def W(name):
            o, w = off[name]
            return wsb[:, o : o + w]

        eps = pp.tile([P, 1], F32, name="eps")
        nc.vector.memset(eps[:], 1e-24)

        xt_l3 = pp.tile([P, 8, BL], F32, name="xt_l3")   # l3 maxes
        xs = pp.tile([P, 8, BL], F32, name="xs")         # l3max + x2max
        xm1 = pp.tile([P, 2, BL], F32, name="xm1")       # branch1 mlp max
        xm2 = pp.tile([P, 4, BL], F32, name="xm2")       # branch2 mlp max

        # ------------------------------------------------------------------
        # memory-addressing branches (tiny; overlaps the big DMA stream)
        # ------------------------------------------------------------------
        with ExitStack() as bctx:
            brp = bctx.enter_context(tc.tile_pool(name="brp", bufs=2, space="PSUM"))
            brs = bctx.enter_context(tc.tile_pool(name="brs", bufs=3))
            for bi, (mfd, M1, M2, xm) in enumerate(
                [(mf1d, 128, 256, xm1), (mf2d, 256, 512, xm2)]
            ):
                S = pp.tile([P, ROWS], F32, name=f"S{bi}")
                nc.gpsimd.dma_start(S[0:CM, :], mfd)
                # squares on partitions 64..127 so one K=128 matmul gives
                # logits and sum-of-squares together
                nc.sync.dma_start(S[CM : 2 * CM, :], S[0:CM, :])
                nc.scalar.square(S[CM : 2 * CM, :], S[CM : 2 * CM, :])

                for g in range(NGROUP):
                    aTp = brp.tile([16, 512], F32, name="aTp", tag="aTp")
                    for chn in range(4):
                        i = g * 4 + chn
                        lss = brp.tile([P, 17], F32, name="lss", tag="lss")
                        nc.tensor.matmul(
                            lss[:],
                            lhsT=S[:, i * P : (i + 1) * P],
                            rhs=W("rhs2"),
                            start=True,
                            stop=True,
                        )
                        r = brs.tile([P, 1], F32, name="rr", tag="rr")
                        nc.scalar.activation(r[:], lss[:, 16:17], AF.Sqrt, bias=eps[:])
                        rinv = brs.tile([P, 1], F32, name="rinv", tag="rinv")
                        nc.vector.reciprocal(rinv[:], r[:])
                        z = brs.tile([P, 16], F32, name="zz", tag="zz")
                        nc.vector.tensor_scalar(z[:], lss[:, 0:16], rinv[:], None, ALU.mult)
                        nm = brs.tile([P, 1], F32, name="nm", tag="nm")
                        nc.vector.tensor_reduce(nm[:], z[:], axis=AX.X, op=ALU.max, negate=True)
                        e = brs.tile([P, 16], F32, name="ee", tag="ee")
                        se = brs.tile([P, 1], F32, name="se", tag="se")
                        nc.scalar.activation(e[:], z[:], AF.Exp, bias=nm[:], accum_out=se[:])
                        rs = brs.tile([P, 1], F32, name="rs", tag="rs")
                        nc.vector.reciprocal(rs[:], se[:])
                        a = brs.tile([P, 16], F32, name="aa", tag="aa")
                        nc.vector.tensor_scalar(a[:], e[:], rs[:], None, ALU.mult)
                        nc.tensor.transpose(aTp[:, chn * P : (chn + 1) * P], a[:], W("ident"))
                    aT = brs.tile([16, 512], F32, name="aT", tag="aT")
                    nc.scalar.copy(aT[:], aTp[:])

                    y1 = brs.tile([P, M1 // P, 512], F32, name="y1", tag="y1")
                    for mj in range(M1 // P):
                        y1p = brp.tile([P, 512], F32, name="y1p", tag="y1p")
                        nc.tensor.matmul(
                            y1p[:],
                            lhsT=W(f"w1eT_b{bi + 1}")[0:16, mj * P : (mj + 1) * P],
                            rhs=aT[:],
                            start=True,
                            stop=True,
                        )
                        nc.scalar.activation(
                            y1[:, mj, :], y1p[:], AF.Relu,
                            bias=W(f"b1_b{bi + 1}")[:, mj : mj + 1],
                        )
                    for mj2 in range(M2 // P):
                        y2p = brp.tile([P, 512], F32, name="y2p", tag="y2p")
                        for kc in range(M1 // P):
                            nc.tensor.matmul(
                                y2p[:],
                                lhsT=W(f"w2T_b{bi + 1}")[:, kc * M2 + mj2 * P : kc * M2 + (mj2 + 1) * P],
                                rhs=y1[:, kc, :],
                                start=(kc == 0),
                                stop=(kc == M1 // P - 1),
                            )
                        y2 = brs.tile([P, 512], F32, name="y2", tag="y2")
                        nc.scalar.activation(
                            y2[:], y2p[:], AF.Relu,
                            bias=W(f"b2_b{bi + 1}")[:, mj2 : mj2 + 1],
                        )
                        nc.vector.tensor_reduce(
                            xm[:, mj2, g * 16 : (g + 1) * 16],
                            y2.rearrange("p (b n) -> p b n", n=NM),
                            axis=AX.X,
                            op=ALU.max,
                        )

        # ------------------------------------------------------------------
        # big maxpools: l3_points then x2_points, 2 batches per DMA
        # ------------------------------------------------------------------
        with ExitStack() as gctx:
            lp = gctx.enter_context(tc.tile_pool(name="lp", bufs=3))
            xp = gctx.enter_context(tc.tile_pool(name="xp", bufs=3))
            tp = gctx.enter_context(tc.tile_pool(name="tp", bufs=4))
            for bp in range(BL // 2):
                t = lp.tile([P, 2, 8, 128], F32, name="l3t", tag="l3t")
                nc.sync.dma_start(
                    t[:], l3d[2 * bp : 2 * bp + 2].rearrange("b (q j) c -> q b j c", j=8)
                )
                nc.vector.tensor_reduce(
                    xt_l3[:, :, 2 * bp : 2 * bp + 2].rearrange("p j b -> p b j"),
                    t[:],
                    axis=AX.X,
                    op=ALU.max,
                )
            for bp in range(BL // 2):
                t = xp.tile([P, 2, 8, 256], F32, name="x2t", tag="x2t")
                nc.sync.dma_start(
                    t[:], x2d[2 * bp : 2 * bp + 2].rearrange("b (q j) c -> q b j c", j=8)
                )
                tm = tp.tile([P, 2, 8], F32, name="tm", tag="tm")
                nc.vector.tensor_reduce(tm[:], t[:], axis=AX.X, op=ALU.max)
                nc.vector.tensor_tensor(
                    xs[:, :, 2 * bp : 2 * bp + 2].rearrange("p j b -> p b j"),
                    tm[:],
                    xt_l3[:, :, 2 * bp : 2 * bp + 2].rearrange("p j b -> p b j"),
                    ALU.add,
                )

        # ------------------------------------------------------------------
        # heads
        # ------------------------------------------------------------------
        with ExitStack() as hctx:
            hp = hctx.enter_context(tc.tile_pool(name="hp", bufs=2, space="PSUM"))
            hs = hctx.enter_context(tc.tile_pool(name="hs", bufs=2))
            for hi, (xmh, pts, npref, odram) in enumerate(
                [(xm1, xt_l3, 2, o1d), (xm2, xs, 4, o2d)]
            ):
                rhs_chunks = [xmh[:, j, :] for j in range(npref)] + [
                    pts[:, j, :] for j in range(8)
                ]
                nk1 = len(rhs_chunks)
                h1 = hs.tile([P, 4, BL], F32, name=f"h1_{hi}", tag="h1")
                for mj in range(4):
                    pp1 = hp.tile([P, BL], F32, name="pp1", tag="pp1")
                    for kc in range(nk1):
                        nc.tensor.matmul(
                            pp1[:],
                            lhsT=W(f"fw1_h{hi + 1}")[:, kc * 512 + mj * P : kc * 512 + (mj + 1) * P],
                            rhs=rhs_chunks[kc],
                            start=(kc == 0),
                            stop=(kc == nk1 - 1),
                        )
                    nc.scalar.activation(
                        h1[:, mj, :], pp1[:], AF.Relu,
                        bias=W(f"fb1_h{hi + 1}")[:, mj : mj + 1],
                    )
                h2 = hs.tile([P, 2, BL], F32, name=f"h2_{hi}", tag="h2")
                for mj in range(2):
                    pp2 = hp.tile([P, BL], F32, name="pp2", tag="pp1")
                    for kc in range(4):
                        nc.tensor.matmul(
                            pp2[:],
                            lhsT=W(f"fw2_h{hi + 1}")[:, kc * 256 + mj * P : kc * 256 + (mj + 1) * P],
                            rhs=h1[:, kc, :],
                            start=(kc == 0),
                            stop=(kc == 3),
                        )
                    nc.scalar.activation(
                        h2[:, mj, :], pp2[:], AF.Relu,
                        bias=W(f"fb2_h{hi + 1}")[:, mj : mj + 1],
                    )
                pp3 = hp.tile([40, BL], F32, name="pp3", tag="pp3")
                for kc in range(2):
                    nc.tensor.matmul(
                        pp3[:],
                        lhsT=W(f"fw3_h{hi + 1}")[:, kc * 40 : (kc + 1) * 40],
                        rhs=h2[:, kc, :],
                        start=(kc == 0),
                        stop=(kc == 1),
                    )
                f3 = hs.tile([40, BL], F32, name=f"f3_{hi}", tag="f3")
                nc.scalar.activation(
                    f3[:], pp3[:], AF.Identity, bias=W(f"fb3_h{hi + 1}")[0:40, 0:1]
                )
                # log_softmax over the 40 classes: transpose to [B, 40]
                zp = hp.tile([BL, 40], F32, name="zp", tag="zp")
                nc.tensor.transpose(zp[:], f3[:], W("ident")[0:40, 0:40])
                z = hs.tile([BL, 40], F32, name=f"z_{hi}", tag="z")
                nc.scalar.copy(z[:], zp[:])
                nm = hs.tile([BL, 1], F32, name="hnm", tag="hnm")
                nc.vector.tensor_reduce(nm[:], z[:], axis=AX.X, op=ALU.max, negate=True)
                e = hs.tile([BL, 40], F32, name="he", tag="he")
                se = hs.tile([BL, 1], F32, name="hse", tag="hse")
                nc.scalar.activation(e[:], z[:], AF.Exp, bias=nm[:], accum_out=se[:])
                lse = hs.tile([BL, 1], F32, name="lse", tag="lse")
                nc.scalar.activation(lse[:], se[:], AF.Ln)
                oo = hs.tile([BL, 40], F32, name=f"oo_{hi}", tag="oo")
                nc.vector.tensor_scalar(oo[:], z[:], nm[:], lse[:], ALU.add, ALU.subtract)
                nc.sync.dma_start(odram, oo[:])

    nc.compile()
    return nc


# ----------------------------------------------------------------------------
# entry point
# ----------------------------------------------------------------------------
_CACHE = {}


def _prep(inputs):
    f = {k: np.ascontiguousarray(np.asarray(v), dtype=np.float32) for k, v in inputs.items()}
    wp, off = _fold_and_pack(f)
    if "nc" not in _CACHE:
        _CACHE["nc"] = _build(off, wp.shape[1])
    in_maps = []
    for c in range(NCORES):
        sl = slice(c * BL, (c + 1) * BL)
        in_maps.append(
            {
                "l3": np.ascontiguousarray(f["l3_points"][sl]),
                "x2": np.ascontiguousarray(f["x2_points"][sl]),
                "mf1": np.ascontiguousarray(
                    np.transpose(f["mem_f1"][sl], (1, 0, 2)).reshape(CM, ROWS)
                ),
                "mf2": np.ascontiguousarray(
                    np.transpose(f["mem_f2"][sl], (1, 0, 2)).reshape(CM, ROWS)
                ),
                "wp": wp,
            }
        )
    return _CACHE["nc"], in_maps


def _run(inputs, trace=False):
    nc, in_maps = _prep(inputs)
    res = run_bass_kernel_spmd(nc, in_maps, core_ids=list(range(NCORES)), trace=trace)
    out1 = np.concatenate([res.results[c]["out1"] for c in range(NCORES)], axis=0)
    out2 = np.concatenate([res.results[c]["out2"] for c in range(NCORES)], axis=0)
    return (out1, out2), res


def kernel(**inputs):
    (out1, out2), _ = _run(inputs, trace=bool(os.environ.get("KERNEL_TRACE")))
    return out1, out2



# revision 6
# speedup vs baseline: 1.6002x; 1.6002x over previous
# Bass/Trainium2 kernel for nn_MENet (scatter_memory).
#
# Strategy: pure data parallel over batch (512 -> 64 per core, 8 cores).
# Host pre-folds BN scales into weights, fuses mlp_w1 @ memory_w.T (so the
# [B,64,32] memory read-out is never materialized), permutes fc1 weight
# columns to match the on-chip maxpool layout, and packs weights into two
# SBUF-layout tensors: wpf (f32: identity/branch weights/biases) and wph
# (bf16: the big head FC weights).
#
# On chip (per core), everything is one software-pipelined stream:
#   - x2_points tiles stream on the sync (SP-HWDGE) DMA queue, l3_points
#     tiles on the scalar (ACT-HWDGE) queue, weights/mem features on the
#     gpsimd (SWDGE) queue -> three queues overlap, HBM stays saturated.
#   - per 2-batch step: DVE max-reduces the l3 tile into xt and the x2
#     tile + add into xs.
#   - memory-addressing branches are emitted in column layout between
#     stream steps: ONE K=128 matmul per 512 rows yields logits(16 rows)
#     + sum-of-squares(row 16); softmax normalizers via exp(-0.5*ln) /
#     exp(-ln) on ACT (no Sqrt -> single activation table for the whole
#     kernel); row-broadcasts via tiny K=1 matmuls; attention feeds the
#     fused MLP; relu folded into the final max-reduce.
#   - heads: bf16 PE matmuls (K-chunks of 128) + ACT bias/ReLU;
#     log_softmax via PE transpose + Exp-with-accum + Ln.
import os
from contextlib import ExitStack

import numpy as np
import ml_dtypes

import concourse.bacc as bacc
import concourse.bass as bass
import concourse.tile as tile
from concourse import mybir
from concourse.bass_utils import run_bass_kernel_spmd

F32 = mybir.dt.float32
BF16 = mybir.dt.bfloat16
AF = mybir.ActivationFunctionType
ALU = mybir.AluOpType
AX = mybir.AxisListType

P = 128
NCORES = 8
B = 512
BL = B // NCORES          # 64 batches per core
NM = 32                   # n points per memory block
CM = 64                   # memory channel dim
ROWS = BL * NM            # 2048 rows per core per branch
NGROUP = ROWS // 512      # 4 groups of 512 rows (16 batches each)
EPS_BN = 1e-5
NSTEP = BL // 2           # 32 stream steps, 2 batches each


# ----------------------------------------------------------------------------
# host-side weight folding + packing
# ----------------------------------------------------------------------------
class _Pack:
    def __init__(self, np_dtype):
        self.parts = []
        self.off = {}
        self.pos = 0
        self.np_dtype = np_dtype

    def add(self, name, arr):
        arr = np.asarray(np.asarray(arr, np.float32), self.np_dtype)
        assert arr.ndim == 2 and arr.shape[0] <= P
        buf = np.zeros((P, arr.shape[1]), self.np_dtype)
        buf[: arr.shape[0]] = arr
        self.off[name] = (self.pos, arr.shape[1])
        self.pos += arr.shape[1]
        self.parts.append(buf)

    def finish(self):
        return np.ascontiguousarray(np.concatenate(self.parts, axis=1))


def _perm_pts(npref, npts):
    # device x-vector position npref + j*128 + q  <-  original point 8q + j
    d = np.arange(npts)
    src = npref + 8 * (d % 128) + (d // 128)
    return np.concatenate([np.arange(npref), src])


def _kpack(w_t):  # [K, M] -> [128, nk, M] flattened to [128, nk*M]
    K, M = w_t.shape
    nk = K // P
    return np.ascontiguousarray(
        np.transpose(w_t.reshape(nk, P, M), (1, 0, 2)).reshape(P, nk * M)
    )


def _fold_and_pack(f):
    s = lambda g: g / np.sqrt(1.0 + EPS_BN)
    mw = f["memory_w"]                                    # [16, 64]
    mn = mw / np.maximum(np.linalg.norm(mw, axis=1, keepdims=True), 1e-12)

    pk = _Pack(np.float32)
    pk.add("ident", np.eye(P, dtype=np.float32))

    rhs2a = np.zeros((P, 16), np.float32)
    rhs2a[0:CM, :] = mn.T                                 # logits part
    pk.add("rhs2a", rhs2a)
    rhs2b = np.zeros((P, 1), np.float32)
    rhs2b[CM : 2 * CM, 0] = 1.0                           # sum-of-squares part
    pk.add("rhs2b", rhs2b)
    pk.add("ones16", np.ones((16, 16), np.float32))
    pk.add("eps", np.full((1, 1), 1e-24, np.float32))

    # branch mlps (conv 1x1): fold BN scale into weights, fuse layer1 with
    # memory_w read-out:  y1[o, row] = sum_s W1e[o, s] * a[row, s]
    for bi, (w1, g1, b1, w2, g2, b2) in enumerate(
        [
            (f["mlp1_w1"], f["mlp1_g1"], f["mlp1_b1"], f["mlp1_w2"], f["mlp1_g2"], f["mlp1_b2"]),
            (f["mlp2_w1"], f["mlp2_g1"], f["mlp2_b1"], f["mlp2_w2"], f["mlp2_g2"], f["mlp2_b2"]),
        ]
    ):
        w1e = (s(g1)[:, None] * w1) @ mw.T                # [M1, 16]
        w2f = s(g2)[:, None] * w2                         # [M2, M1]
        M1, M2 = w2f.shape[1], w2f.shape[0]
        pk.add(f"w1eT_b{bi + 1}", w1e.T)                  # [16, M1]
        pk.add(f"b1_b{bi + 1}", b1.reshape(M1 // P, P).T) # [128, M1/128]
        pk.add(f"w2T_b{bi + 1}", _kpack(w2f.T))           # [128, (M1/128)*M2]
        pk.add(f"b2_b{bi + 1}", b2.reshape(M2 // P, P).T)

    # heads: fold BN into fc1/fc2, permute fc1 cols for the maxpool layout.
    # Big weights go to the bf16 pack, biases stay f32.
    ph = _Pack(ml_dtypes.bfloat16)
    for hi, (w1, b1, g1, bb1, w2, b2, g2, bb2, w3, b3, npref) in enumerate(
        [
            (f["fc1_w"], f["fc1_b"], f["bn1_g"], f["bn1_b"], f["fc2_w"], f["fc2_b"],
             f["bn2_g"], f["bn2_b"], f["fc3_w"], f["fc3_b"], 256),
            (f["fc1_2_w"], f["fc1_2_b"], f["bn1_2_g"], f["bn1_2_b"], f["fc2_2_w"],
             f["fc2_2_b"], f["bn2_2_g"], f["bn2_2_b"], f["fc3_2_w"], f["fc3_2_b"], 512),
        ]
    ):
        s1, s2 = s(g1), s(g2)
        w1f = (s1[:, None] * w1)[:, _perm_pts(npref, 1024)]   # [512, npref+1024]
        b1f = s1 * b1 + bb1
        w2f = s2[:, None] * w2                                # [256, 512]
        b2f = s2 * b2 + bb2
        ph.add(f"fw1_h{hi + 1}", _kpack(w1f.T))               # [128, nk1*512]
        pk.add(f"fb1_h{hi + 1}", b1f.reshape(4, P).T)
        ph.add(f"fw2_h{hi + 1}", _kpack(w2f.T))               # [128, 4*256]
        pk.add(f"fb2_h{hi + 1}", b2f.reshape(2, P).T)
        ph.add(f"fw3_h{hi + 1}", _kpack(w3.T))                # [128, 2*40]
        pk.add(f"fb3_h{hi + 1}", b3.reshape(40, 1))

    return pk.finish(), pk.off, ph.finish(), ph.off


# ----------------------------------------------------------------------------
# device program
# ----------------------------------------------------------------------------
def _build(offf, NWF, offh, NWH):
    nc = bacc.Bacc("TRN2", target_bir_lowering=False, debug=False)
    l3d = nc.dram_tensor("l3", [BL, 1024, 128], F32, kind="ExternalInput").ap()
    x2d = nc.dram_tensor("x2", [BL, 1024, 256], F32, kind="ExternalInput").ap()
    mf1d = nc.dram_tensor("mf1", [CM, ROWS], F32, kind="ExternalInput").ap()
    mf2d = nc.dram_tensor("mf2", [CM, ROWS], F32, kind="ExternalInput").ap()
    wpfd = nc.dram_tensor("wpf", [P, NWF], F32, kind="ExternalInput").ap()
    wphd = nc.dram_tensor("wph", [P, NWH], BF16, kind="ExternalInput").ap()
    o1d = nc.dram_tensor("out1", [BL, 40], F32, kind="ExternalOutput").ap()
    o2d = nc.dram_tensor("out2", [BL, 40], F32, kind="ExternalOutput").ap()

    with tile.TileContext(nc) as tc, ExitStack() as ctx:
        pp = ctx.enter_context(tc.tile_pool(name="persist", bufs=1))
        wsf = pp.tile([P, NWF], F32, name="wsf")
        wsh = pp.tile([P, NWH], BF16, name="wsh")
        S1 = pp.tile([P, ROWS], F32, name="S1")
        S2 = pp.tile([P, ROWS], F32, name="S2")

        # gpsimd (SWDGE) queue: small/critical loads first, big head weights
        # last.  Squares live on partitions 64..127 so one K=128 matmul gives
        # logits and sum-of-squares together.
        nc.gpsimd.dma_start(S1[0:CM, :], mf1d)
        nc.gpsimd.dma_start(S1[CM : 2 * CM, :], S1[0:CM, :])
        nc.gpsimd.dma_start(S2[0:CM, :], mf2d)
        nc.gpsimd.dma_start(S2[CM : 2 * CM, :], S2[0:CM, :])
        nc.gpsimd.dma_start(wsf[:], wpfd)
        nc.gpsimd.dma_start(wsh[:], wphd)

        def Wf(name):
            o, w = offf[name]
            return wsf[:, o : o + w]

        def Wh(name):
            o, w = offh[name]
            return wsh[:, o : o + w]

        xt32 = pp.tile([P, 8, BL], F32, name="xt32")     # l3 maxes
        xs32 = pp.tile([P, 8, BL], F32, name="xs32")     # l3max + x2max
        xtb = pp.tile([P, 8, BL], BF16, name="xtb")
        xsb = pp.tile([P, 8, BL], BF16, name="xsb")
        xm1 = pp.tile([P, 2, BL], BF16, name="xm1")      # branch1 mlp max
        xm2 = pp.tile([P, 4, BL], BF16, name="xm2")      # branch2 mlp max

        with ExitStack() as bctx:
            brp1 = bctx.enter_context(tc.tile_pool(name="brp1", bufs=1, space="PSUM"))
            brp2 = bctx.enter_context(tc.tile_pool(name="brp2", bufs=2, space="PSUM"))
            brs = bctx.enter_context(tc.tile_pool(name="brs", bufs=2))
            lp = bctx.enter_context(tc.tile_pool(name="lp", bufs=4))
            xp = bctx.enter_context(tc.tile_pool(name="xp", bufs=4))
            tp = bctx.enter_context(tc.tile_pool(name="tp", bufs=3))

            def emit_unit(bi, g):
                # memory addressing for 512 rows (16 batches) in column
                # layout: rows of lss = 16 logits + sum-of-squares.
                S = S1 if bi == 0 else S2
                M1, M2 = (128, 256) if bi == 0 else (256, 512)
                xm = xm1 if bi == 0 else xm2
                lss = brp1.tile([16, 512], F32, name="lss", tag="lss")
                nc.tensor.matmul(
                    lss[:], lhsT=Wf("rhs2a")[:, 0:16], rhs=S[:, g * 512 : (g + 1) * 512],
                    start=True, stop=True,
                )
                ssp = brp1.tile([1, 512], F32, name="ssp", tag="ssp")
                nc.tensor.matmul(
                    ssp[:], lhsT=Wf("rhs2b")[:, 0:1], rhs=S[:, g * 512 : (g + 1) * 512],
                    start=True, stop=True,
                )
                # 1/||x|| = exp(-0.5*ln(ss)); 1/sum(e) = exp(-ln(v)).  Both
                # stay on the exp/ln activation table (no Sqrt anywhere).
                lnss = brs.tile([1, 512], F32, name="lnss", tag="lnss")
                nc.scalar.activation(lnss[:], ssp[0:1, :], AF.Ln, bias=Wf("eps")[0:1, 0:1])
                rinv = brs.tile([1, 512], F32, name="rinv", tag="rinv")
                nc.scalar.activation(rinv[:], lnss[:], AF.Exp, scale=-0.5)
                rb = brp1.tile([16, 512], F32, name="rb", tag="rb")
                nc.tensor.matmul(rb[:], lhsT=Wf("ones16")[0:1, :], rhs=rinv[:], start=True, stop=True)
                lssS = brs.tile([16, 512], F32, name="lssS", tag="lssS")
                nc.scalar.activation(lssS[:], lss[:], AF.Identity)
                z = brs.tile([16, 512], F32, name="z", tag="z")
                nc.vector.tensor_tensor(z[:], lssS[:], rb[:], ALU.mult)
                # |z| <= 1 by Cauchy-Schwarz, so exp needs no max-shift
                e = brs.tile([16, 512], F32, name="e", tag="e")
                nc.scalar.activation(e[:], z[:], AF.Exp)
                v = brp1.tile([16, 512], F32, name="v", tag="v")
                nc.tensor.matmul(v[:], lhsT=Wf("ones16")[0:16, :], rhs=e[:], start=True, stop=True)
                lnv = brs.tile([1, 512], F32, name="lnv", tag="lnss")
                nc.scalar.activation(lnv[:], v[0:1, :], AF.Ln)
                rv = brs.tile([1, 512], F32, name="rv", tag="rinv")
                nc.scalar.activation(rv[:], lnv[:], AF.Exp, scale=-1.0)
                rvb = brp1.tile([16, 512], F32, name="rvb", tag="rb")
                nc.tensor.matmul(rvb[:], lhsT=Wf("ones16")[0:1, :], rhs=rv[:], start=True, stop=True)
                a = brs.tile([16, 512], F32, name="a", tag="a")
                nc.vector.tensor_tensor(a[:], e[:], rvb[:], ALU.mult)

                y1 = brs.tile([P, M1 // P, 512], F32, name="y1", tag=f"y1b{bi}")
                for mj in range(M1 // P):
                    y1p = brp2.tile([P, 512], F32, name="y1p", tag="y1p")
                    nc.tensor.matmul(
                        y1p[:], lhsT=Wf(f"w1eT_b{bi + 1}")[0:16, mj * P : (mj + 1) * P],
                        rhs=a[:], start=True, stop=True,
                    )
                    nc.scalar.activation(
                        y1[:, mj, :], y1p[:], AF.Relu,
                        bias=Wf(f"b1_b{bi + 1}")[:, mj : mj + 1],
                    )
                for mj2 in range(M2 // P):
                    y2p = brp2.tile([P, 512], F32, name="y2p", tag="y2p")
                    for kc in range(M1 // P):
                        nc.tensor.matmul(
                            y2p[:],
                            lhsT=Wf(f"w2T_b{bi + 1}")[:, kc * M2 + mj2 * P : kc * M2 + (mj2 + 1) * P],
                            rhs=y1[:, kc, :],
                            start=(kc == 0),
                            stop=(kc == M1 // P - 1),
                        )
                    # max_n(relu(u + b2)) = relu(max_n(u) + b2): reduce raw
                    # PSUM, then one tiny bias+relu (+ bf16 cast) per block.
                    t16 = brs.tile([P, 16], F32, name="t16", tag="t16")
                    nc.vector.tensor_reduce(
                        t16[:], y2p.rearrange("p (b n) -> p b n", n=NM),
                        axis=AX.X, op=ALU.max,
                    )
                    nc.scalar.activation(
                        xm[:, mj2, g * 16 : (g + 1) * 16], t16[:], AF.Relu,
                        bias=Wf(f"b2_b{bi + 1}")[:, mj2 : mj2 + 1],
                    )

            units = [(bi, g) for g in range(NGROUP) for bi in (0, 1)]
            unit_at = {2 + 3 * i: u for i, u in enumerate(units)}

            # ----------------------------------------------------------------
            # main stream: 2 batches per step; x2 on sync queue, l3 on the
            # scalar queue, branch units interleaved every 3rd step.
            # ----------------------------------------------------------------
            for bp in range(NSTEP):
                xtile = xp.tile([P, 2, 8, 256], F32, name="x2t", tag="x2t")
                nc.sync.dma_start(
                    xtile[:], x2d[2 * bp : 2 * bp + 2].rearrange("b (q j) c -> q b j c", j=8)
                )
                ltile = lp.tile([P, 2, 8, 128], F32, name="l3t", tag="l3t")
                nc.scalar.dma_start(
                    ltile[:], l3d[2 * bp : 2 * bp + 2].rearrange("b (q j) c -> q b j c", j=8)
                )
                if bp == 1:
                    nc.scalar.activation(S1[CM : 2 * CM, :], S1[CM : 2 * CM, :], AF.Square)
                    nc.scalar.activation(S2[CM : 2 * CM, :], S2[CM : 2 * CM, :], AF.Square)
                nc.vector.tensor_reduce(
                    xt32[:, :, 2 * bp : 2 * bp + 2].rearrange("p j b -> p b j"),
                    ltile[:], axis=AX.X, op=ALU.max,
                )
                tm = tp.tile([P, 2, 8], F32, name="tm", tag="tm")
                nc.vector.tensor_reduce(tm[:], xtile[:], axis=AX.X, op=ALU.max)
                nc.vector.tensor_tensor(
                    xs32[:, :, 2 * bp : 2 * bp + 2].rearrange("p j b -> p b j"),
                    tm[:],
                    xt32[:, :, 2 * bp : 2 * bp + 2].rearrange("p j b -> p b j"),
                    ALU.add,
                )
                if bp in unit_at:
                    emit_unit(*unit_at[bp])

            # bf16 copies of the pooled features for the bf16 head matmuls
            nc.vector.tensor_copy(xtb.rearrange("p j b -> p (j b)"), xt32.rearrange("p j b -> p (j b)"))
            nc.vector.tensor_copy(xsb.rearrange("p j b -> p (j b)"), xs32.rearrange("p j b -> p (j b)"))

        # ------------------------------------------------------------------
        # heads
        # ------------------------------------------------------------------
        with ExitStack() as hctx:
            hp = hctx.enter_context(tc.tile_pool(name="hp", bufs=2, space="PSUM"))
            hs = hctx.enter_context(tc.tile_pool(name="hs", bufs=2))
            for hi, (xmh, npref, pts, odram) in enumerate(
                [(xm1, 2, xtb, o1d), (xm2, 4, xsb, o2d)]
            ):
                rhs_chunks = [xmh[:, j, :] for j in range(npref)] + [
                    pts[:, j, :] for j in range(8)
                ]
                nk1 = len(rhs_chunks)
                h1 = hs.tile([P, 4, BL], BF16, name=f"h1_{hi}", tag="h1")
                for mj in range(4):
                    pp1 = hp.tile([P, BL], F32, name="pp1", tag="pp1")
                    for kc in range(nk1):
                        nc.tensor.matmul(
                            pp1[:],
                            lhsT=Wh(f"fw1_h{hi + 1}")[:, kc * 512 + mj * P : kc * 512 + (mj + 1) * P],
                            rhs=rhs_chunks[kc],
                            start=(kc == 0),
                            stop=(kc == nk1 - 1),
                        )
                    nc.scalar.activation(
                        h1[:, mj, :], pp1[:], AF.Relu,
                        bias=Wf(f"fb1_h{hi + 1}")[:, mj : mj + 1],
                    )
                h2 = hs.tile([P, 2, BL], BF16, name=f"h2_{hi}", tag="h2")
                for mj in range(2):
                    pp2 = hp.tile([P, BL], F32, name="pp2", tag="pp1")
                    for kc in range(4):
                        nc.tensor.matmul(
                            pp2[:],
                            lhsT=Wh(f"fw2_h{hi + 1}")[:, kc * 256 + mj * P : kc * 256 + (mj + 1) * P],
                            rhs=h1[:, kc, :],
                            start=(kc == 0),
                            stop=(kc == 3),
                        )
                    nc.scalar.activation(
                        h2[:, mj, :], pp2[:], AF.Relu,
                        bias=Wf(f"fb2_h{hi + 1}")[:, mj : mj + 1],
                    )
                pp3 = hp.tile([40, BL], F32, name="pp3", tag="pp3")
                for kc in range(2):
                    nc.tensor.matmul(
                        pp3[:],
                        lhsT=Wh(f"fw3_h{hi + 1}")[:, kc * 40 : (kc + 1) * 40],
                        rhs=h2[:, kc, :],
                        start=(kc == 0),
                        stop=(kc == 1),
                    )
                f3 = hs.tile([40, BL], F32, name=f"f3_{hi}", tag="f3")
                nc.scalar.activation(
                    f3[:], pp3[:], AF.Identity, bias=Wf(f"fb3_h{hi + 1}")[0:40, 0:1]
                )
                # log_softmax over the 40 classes: transpose to [B, 40]
                zp = hp.tile([BL, 40], F32, name="zp", tag="zp")
                nc.tensor.transpose(zp[:], f3[:], Wf("ident")[0:40, 0:40])
                z = hs.tile([BL, 40], F32, name=f"z_{hi}", tag="z")
                nc.scalar.copy(z[:], zp[:])
                nm = hs.tile([BL, 1], F32, name="hnm", tag="hnm")
                nc.vector.tensor_reduce(nm[:], z[:], axis=AX.X, op=ALU.max, negate=True)
                e = hs.tile([BL, 40], F32, name="he", tag="he")
                se = hs.tile([BL, 1], F32, name="hse", tag="hse")
                nc.scalar.activation(e[:], z[:], AF.Exp, bias=nm[:], accum_out=se[:])
                lse = hs.tile([BL, 1], F32, name="lse", tag="lse")
                nc.scalar.activation(lse[:], se[:], AF.Ln)
                oo = hs.tile([BL, 40], F32, name=f"oo_{hi}", tag="oo")
                nc.vector.tensor_scalar(oo[:], z[:], nm[:], lse[:], ALU.add, ALU.subtract)
                nc.sync.dma_start(odram, oo[:])

    nc.compile()
    return nc


# ----------------------------------------------------------------------------
# entry point
# ----------------------------------------------------------------------------
_CACHE = {}


def _prep(inputs):
    f = {k: np.ascontiguousarray(np.asarray(v), dtype=np.float32) for k, v in inputs.items()}
    wpf, offf, wph, offh = _fold_and_pack(f)
    if "nc" not in _CACHE:
        _CACHE["nc"] = _build(offf, wpf.shape[1], offh, wph.shape[1])
    in_maps = []
    for c in range(NCORES):
        sl = slice(c * BL, (c + 1) * BL)
        in_maps.append(
            {
                "l3": np.ascontiguousarray(f["l3_points"][sl]),
                "x2": np.ascontiguousarray(f["x2_points"][sl]),
                "mf1": np.ascontiguousarray(
                    np.transpose(f["mem_f1"][sl], (1, 0, 2)).reshape(CM, ROWS)
                ),
                "mf2": np.ascontiguousarray(
                    np.transpose(f["mem_f2"][sl], (1, 0, 2)).reshape(CM, ROWS)
                ),
                "wpf": wpf,
                "wph": wph,
            }
        )
    return _CACHE["nc"], in_maps


def _run(inputs, trace=False):
    nc, in_maps = _prep(inputs)
    res = run_bass_kernel_spmd(nc, in_maps, core_ids=list(range(NCORES)), trace=trace)
    out1 = np.concatenate([res.results[c]["out1"] for c in range(NCORES)], axis=0)
    out2 = np.concatenate([res.results[c]["out2"] for c in range(NCORES)], axis=0)
    return (out1, out2), res


def kernel(**inputs):
    (out1, out2), _ = _run(inputs, trace=bool(os.environ.get("KERNEL_TRACE")))
    return out1, out2
